# revision 18
# baseline (speedup 1.0000x reference)
"""Dot-product attention on 8 Trainium2 NeuronCores.

Full inputs [B=4, H=16, S=1024, D=64] fp32. B*H = 64 heads are sharded
8-per-core (head parallel), processed in head PAIRS in a 3-deep software
pipeline: phase p runs MM1+exp of pair p, MM2 (+sums reciprocal) of pair
p-1, and normalize+store of pair p-2 concurrently.

Per head pair on-device:
  scores^T[k,q] = K d-major @ Q d-major  (bf16, K=64 contraction, the two
                                          heads row-packed into PE halves;
                                          2 ping-ponged [128,1024] PSUM
                                          stage tiles so the exp stream
                                          never stalls MM1)
  E = exp(scores^T / sqrt(d_k))          (ScalarE PSUM->SBUF, bf16 out,
                                          [128,1024] chunks — the ScalarE
                                          exp stream is the wall)
  outT+sums     = [V | 1]^T @ E          (bf16 SPLIT-K: even/odd k-halves
                                          as two concurrent K=64 row-group
                                          matmuls into separate PSUM banks,
                                          summed on DVE; ones column gives
                                          softmax sums in row 64)
  r = 1/sums                             (DVE reciprocal, scheduled a full
                                          phase ahead of its consumers; the
                                          tail pair uses ScalarE exp(-ln(s))
                                          since ScalarE is idle by then)
  out           = outT * (sel_r^T @ r)   (K=64 fp32r selector matmul
                                          broadcast + fused DVE multiply)
Host side transposes Q/K to d-major bf16 when sharding and un-transposes
the d-major output, both in numpy.

Toolchain notes for this container (walrus 2026-05-04 + bass_rust skew):
 - walrus accepts at most ONE sync-wait per instruction. A JSON pass over
   the BIR inserts NoOps carrying extra waits right before the owning
   instruction (same engine, in-order => semantics preserved). The
   TileContext tail drain is patched the same way.
"""

import json
from contextlib import ExitStack

import numpy as np

import concourse.bass as bass
import concourse.bass2jax as bass2jax
import concourse.mybir as mybir
import concourse.tile as tile
from concourse import bass_utils
from concourse.vector_clock import ScopedClock

F32 = mybir.dt.float32
F32R = mybir.dt.float32r
BF16 = mybir.dt.bfloat16

N_CORES = 8
HEADS_PER_CORE = 8
S = 1024
D = 64
KT = S // 128  # 8 k-tiles per head
NPAIR = HEADS_PER_CORE // 2

_DRAIN_MAX_WAITS = 1


def _split_drain_and_barrier(self, tick_clock, wait_clock):
    nc = self.nc
    drain_inst = nc.sync.drain()
    wait_clock.add_sem_waits(
        drain_inst.ins, ScopedClock({None: tick_clock.global_clock})
    )
    si = drain_inst.ins.sync_info
    if si is not None and si.on_wait and len(si.on_wait) > _DRAIN_MAX_WAITS:
        waits = list(si.on_wait)
        updates = list(si.on_update or [])
        drain_inst.ins.sync_info = mybir.SyncInfo(
            on_wait=waits[:_DRAIN_MAX_WAITS], on_update=[]
        )
        rest = waits[_DRAIN_MAX_WAITS:]
        for i in range(0, len(rest), _DRAIN_MAX_WAITS):
            extra = nc.sync.drain()
            extra.ins.sync_info = mybir.SyncInfo(
                on_wait=rest[i : i + _DRAIN_MAX_WAITS],
                on_update=updates if i + _DRAIN_MAX_WAITS >= len(rest) else [],
            )
    nc.all_engine_barrier()
    assert self.sems is not None
    popped = nc._tile_sem_poison_stack.pop()
    assert popped is self._sem_poison
    nc.clear_and_free_semaphores(list(self.sems.allocated().values()))
    nc.all_engine_barrier()


def _split_waits_in_bir(bir_json: bytes) -> bytes:
    """Hoist extra sync-waits onto NoOps inserted immediately before the
    owning instruction (same engine, in-order => semantics unchanged)."""
    j = json.loads(bir_json)
    n = 0
    for f in j["functions"]:
        for b in f["blocks"]:
            out = []
            for inst in b["instructions"]:
                si = inst.get("sync_info")
                waits = (si or {}).get("on_wait") or []
                if len(waits) > 1:
                    for w in waits[:-1]:
                        out.append(
                            {
                                "debug": inst.get("debug", 0),
                                "engine": inst["engine"],
                                "ins": [],
                                "outs": [],
                                "name": f"{inst['name']}-wsplit{n}",
                                "opcode": "NoOp",
                                "sync_info": {"on_update": [], "on_wait": [w]},
                            }
                        )
                        n += 1
                    si["on_wait"] = [waits[-1]]
                out.append(inst)
            b["instructions"] = out
    return json.dumps(j).encode()


_orig_compile_bir_kernel = bass_utils.compile_bir_kernel


def _compile_bir_kernel_splitting(bir_json, tmpdir, neff_name="file.neff"):
    return _orig_compile_bir_kernel(_split_waits_in_bir(bir_json), tmpdir, neff_name)


# walrus's lower_dve pass crashes on this kernel with ldw-opt enabled
ENABLE_LDW_OPT = False
_orig_run_command = bass_utils.run_command


def _run_command_ldw(argv, **kwargs):
    if ENABLE_LDW_OPT:
        argv = [
            a.replace("--enable-ldw-opt=false", "--enable-ldw-opt=true") for a in argv
        ]
    return _orig_run_command(argv, **kwargs)


def _install_patches():
    if not getattr(tile.TileContext, "_drain_split_installed", False):
        tile.TileContext._drain_and_barrier = _split_drain_and_barrier
        tile.TileContext._drain_split_installed = True
    if bass_utils.compile_bir_kernel is not _compile_bir_kernel_splitting:
        bass_utils.compile_bir_kernel = _compile_bir_kernel_splitting
        bass2jax.compile_bir_kernel = _compile_bir_kernel_splitting
        bass_utils.run_command = _run_command_ldw


# Tail reciprocal on ScalarE as exp(-ln(s)) (Ln and Exp share one act
# table set); mid-pipeline reciprocals run on DVE where they hide under
# the ScalarE exp stream.
TAIL_RECIP_SCALAR = True


def build_nc(scale: float) -> bass.Bass:
    _install_patches()
    nc = bass.Bass(
        trn_type="TRN2", target_bir_lowering=False, debug=False, num_devices=N_CORES
    )
    # kq[pair, 0:64, 0:1024] = Q^T head 2p ; [0:64, 1024:] = K^T head 2p
    # kq[pair, 64:128, ...]  = same for head 2p+1    (d-major, bf16)
    kq = nc.dram_tensor(
        "kq", [NPAIR, 128, 2 * S], BF16, kind="ExternalInput"
    ).ap()
    # vext[h, p, t, j]: V[h, 128*t + p, j] for j < 64, 1.0 at j == 64 (bf16)
    vext = nc.dram_tensor(
        "vext", [HEADS_PER_CORE, 128, KT, 65], BF16, kind="ExternalInput"
    ).ap()
    # sels[r, k, m] = 1.0 where k == 32*r: selector weights that extract and
    # broadcast row 32r of a [128, .] rhs across 64 output partitions.
    sels_d = nc.dram_tensor("sels", [4, 128, D], F32R, kind="ExternalInput").ap()
    outT = nc.dram_tensor(
        "outT", [HEADS_PER_CORE, D, S], F32, kind="ExternalOutput"
    ).ap()

    with tile.TileContext(nc) as tc, ExitStack() as ctx:
        sb = ctx.enter_context(tc.tile_pool(name="sb", bufs=2))
        singles = ctx.enter_context(tc.tile_pool(name="singles", bufs=1))
        ou_pool = ctx.enter_context(tc.tile_pool(name="ou", bufs=8))
        os_pool = ctx.enter_context(tc.tile_pool(name="os", bufs=2))
        # PSUM: 2 ping-pong stage tiles [128,1024] = 2 banks each (4 total);
        # "o" pool shares the other 4 banks between split-k MM2 A/B tiles
        # [65,512] and bcast tiles [64,512] (1 bank each).
        ps_stage = ctx.enter_context(
            tc.tile_pool(name="ps_stage", bufs=2, space="PSUM")
        )
        ps_o = ctx.enter_context(tc.tile_pool(name="ps_o", bufs=4, space="PSUM"))

        state = {}  # pair -> dict(kq, va, vb, e)
        nstate = {}  # pair -> dict(ou={g: tile}, os={half: tile})

        def prefetch(p, first=False):
            kq_s = sb.tile([128, 2 * S], BF16, tag="kq")
            if first:
                # Slots 0-7 run c=1, so Q[512:1024] + k-tile 0 -- one
                # CONTIGUOUS dma (wide lines = fewer descriptors) -- gates
                # the first matmul. K-tiles 1+ go on the scalar ring (k-tile
                # 1 alone first: needed ~1us after the first exp).
                nc.sync.dma_start(kq_s[:, 512 : S + 128], kq[p][:, 512 : S + 128])
                nc.sync.dma_start(kq_s[:, 0:512], kq[p][:, 0:512])
                nc.scalar.dma_start(
                    kq_s[:, S + 128 : S + 384], kq[p][:, S + 128 : S + 384]
                )
                nc.scalar.dma_start(kq_s[:, S + 384 :], kq[p][:, S + 384 :])
            else:
                nc.sync.dma_start(kq_s, kq[p])
            va = sb.tile([128, KT, 65], BF16, tag="va")
            nc.gpsimd.dma_start(va, vext[2 * p])
            vb = sb.tile([128, KT, 65], BF16, tag="vb")
            nc.gpsimd.dma_start(vb, vext[2 * p + 1])
            e_s = sb.tile([128, KT, 2, S], BF16, tag="e")
            state[p] = dict(kq=kq_s, va=va, vb=vb, e=e_s)

        def emit_mm1_exp(p, ki, c):
            st = state[p]
            kq_s = st["kq"]
            stage = ps_stage.tile([128, 1024], F32, tag="stage")
            kslice = slice(S + ki * 128, S + (ki + 1) * 128)
            qslice = slice(c * 512, (c + 1) * 512)
            for half in range(2):
                b = 64 * half
                nc.tensor.matmul(
                    stage[:, half * 512 : (half + 1) * 512],
                    kq_s[b : b + 64, kslice],
                    kq_s[b : b + 64, qslice],
                    start=True,
                    stop=True,
                )
            nc.scalar.activation(
                out=st["e"][:, ki, c, :],
                in_=stage,
                func=mybir.ActivationFunctionType.Exp,
                scale=scale,
            )

        mm2_ps = {}  # (p, g) -> (oa, ob)
        C_FIRST = 1  # c value run in slots 0-7 (chosen so the first-pair
        # critical dma [Q c=1 | k-tile 0] is one contiguous column range)

        def emit_mm2_chunk(p, g, kis, combine=True):
            """Part of a split-k MM2 group (kis = consecutive ki range).
            After the last chunk (if combine), emits the DVE combine. Carry
            groups do the sums rows FIRST so the reciprocal unblocks early."""
            half, c = divmod(g, 2)
            st = state[p]
            v_s = st["va"] if half == 0 else st["vb"]
            e_s = st["e"]
            if kis[0] == 0:
                oa = ps_o.tile([65, 512], F32, tag="o")
                ob = ps_o.tile([65, 512], F32, tag="o")
                mm2_ps[(p, g)] = (oa, ob)
            else:
                oa, ob = mm2_ps[(p, g)]
            qs = slice(half * 512, (half + 1) * 512)
            for ki in kis:
                nc.tensor.matmul(
                    oa, v_s[0:64, ki, :], e_s[0:64, ki, c, qs],
                    start=(ki == 0), stop=(ki == KT - 1),
                )
                nc.tensor.matmul(
                    ob, v_s[64:128, ki, :], e_s[64:128, ki, c, qs],
                    start=(ki == 0), stop=(ki == KT - 1),
                )
            if kis[-1] != KT - 1 or not combine:
                return
            emit_mm2_srow(p, g)
            emit_mm2_ou(p, g)

        def emit_mm2_srow(p, g):
            oa, ob = mm2_ps[(p, g)]
            srow = sums_sp[32 * g : 32 * g + 1, :]
            half, c = divmod(g, 2)
            if c != C_FIRST:
                nc.vector.tensor_copy(srow, oa[64:65, :])
                nc.vector.scalar_tensor_tensor(
                    out=srow, in0=ob[64:65, :], scalar=1.0,
                    op0=mybir.AluOpType.mult, in1=srow, op1=mybir.AluOpType.add,
                )

        def emit_mm2_ou(p, g, scalar_copy=False):
            oa, ob = mm2_ps[(p, g)]
            half, c = divmod(g, 2)
            ou = ou_pool.tile([65, 512], F32, tag="ou")
            if c != C_FIRST:
                # sums row already handled by emit_mm2_srow
                if scalar_copy:
                    nc.scalar.copy(ou[0:64, :], oa[0:64, :])
                else:
                    nc.vector.tensor_copy(ou[0:64, :], oa[0:64, :])
                nc.vector.scalar_tensor_tensor(
                    out=ou[0:64, :], in0=ob[0:64, :], scalar=1.0,
                    op0=mybir.AluOpType.mult, in1=ou[0:64, :],
                    op1=mybir.AluOpType.add,
                )
            else:
                srow = sums_sp[32 * g : 32 * g + 1, :]
                nc.vector.tensor_copy(ou, oa)
                nc.vector.scalar_tensor_tensor(
                    out=ou, in0=ob, scalar=1.0,
                    op0=mybir.AluOpType.mult, in1=ou, op1=mybir.AluOpType.add,
                )
                nc.vector.tensor_copy(srow, ou[64:65, :])
            nstate[p]["ou"][g] = ou

        def emit_recip(p):
            with nc.allow_low_precision(reason="fp32r recip for bcast matmul"):
                nc.vector.reciprocal(out=recip_sp, in_=sums_sp)

        def emit_recip_half(hr):
            # ScalarE is idle after the last exp: r = exp(-ln(s)) per half.
            with nc.allow_low_precision(reason="fp32r recip for bcast matmul"):
                nc.scalar.activation(
                    out=lntmp[hr, :], in_=sums_sp[hr, :],
                    func=mybir.ActivationFunctionType.Ln,
                )
                nc.scalar.activation(
                    out=recip_sp[hr, :], in_=lntmp[hr, :],
                    func=mybir.ActivationFunctionType.Exp, scale=-1.0,
                )

        def emit_normalize(p, g):
            half, c = divmod(g, 2)
            h = 2 * p + half
            hr = slice(0, 64) if g < 2 else slice(64, 128)
            bc = ps_o.tile([D, 512], F32, tag="o")
            nc.tensor.matmul(
                bc, sels_s[hr, g, :], recip_sp[hr, :], start=True, stop=True
            )
            if c == 0:
                o_s = os_pool.tile([D, S], F32, tag=f"os{half}")
                nstate[p]["os"][half] = o_s
            else:
                o_s = nstate[p]["os"][half]
            ou = nstate[p]["ou"][g]
            cs = slice(c * 512, (c + 1) * 512)
            nc.vector.scalar_tensor_tensor(
                out=o_s[:, cs],
                in0=bc,
                scalar=1.0,
                op0=mybir.AluOpType.mult,
                in1=ou[0:64, :],
                op1=mybir.AluOpType.mult,
            )
            nc.sync.dma_start(outT[h][:, cs], o_s[:, cs])

        prefetch(0, first=True)
        # constants are needed only from phase 1 on; issue them after kq(0)
        sels_s = singles.tile([128, 4, D], F32R, tag="sels")
        nc.gpsimd.dma_start(sels_s, sels_d.rearrange("r k m -> k r m"))
        # persistent sums/recip scratch; rows {0,32,64,96} hold live data,
        # the rest stay at 1.0 so the reciprocal never produces non-finites.
        sums_sp = singles.tile([128, 512], F32, tag="sums_sp")
        nc.vector.memset(sums_sp, 1.0)
        recip_sp = singles.tile([128, 512], F32R, tag="recip_sp")
        lntmp = singles.tile([128, 512], F32, tag="lntmp")

        # Phase p slot map (c-major: slots 0-7 = (ki, c=0), 8-15 = (ki, c=1)):
        #   slot 0-3:   MM2 of pair p-1's c=1 groups (g1, g3), half-group
        #               per slot to keep the PE queue smooth
        #   slot 4:     reciprocal of pair p-1's sums (8 slots of slack
        #               before its consumers at slots 12-15)
        #   slot 6:     prefetch pair p+1
        #   slot 8-11:  MM2 of pair p's own c=0 groups (g0, g2) -- their
        #               exps all landed in slots 0-7 of THIS phase
        #   slot 12-15: normalize pair p-1 (bcast+STT), out-DMA per half
        # groups g=(half,c): g0=(0,0) g1=(0,1) g2=(1,0) g3=(1,1)
        NORM_ORDER = (0, 2, 1, 3)
        KI_LO, KI_HI = range(0, 4), range(4, KT)
        # same-phase groups (c == C_FIRST) and carry groups (c != C_FIRST)
        SAME_G = (0 + C_FIRST, 2 + C_FIRST)
        CARRY_G = (1 - C_FIRST, 3 - C_FIRST)
        for p in range(NPAIR):
            nstate[p] = dict(ou={}, os={})
            for s in range(16):
                c = C_FIRST if s < 8 else 1 - C_FIRST
                ki = s % 8
                emit_mm1_exp(p, ki, c)
                if 1 <= p:
                    if s < 4:
                        g = CARRY_G[0] if s < 2 else CARRY_G[1]
                        emit_mm2_chunk(p - 1, g, KI_LO if s % 2 == 0 else KI_HI)
                    elif s == 4:
                        emit_recip(p - 1)
                    elif 12 <= s:
                        emit_normalize(p - 1, NORM_ORDER[s - 12])
                if 8 <= s < 12:
                    # both same-phase groups in quarter chunks per slot so
                    # the PE queue never bunches more than ~0.9us per slot
                    q4 = s - 8
                    for g in SAME_G:
                        emit_mm2_chunk(p, g, range(2 * q4, 2 * q4 + 2))
                if p + 1 < NPAIR and s == 6:
                    prefetch(p + 1)
        # tail: pair NPAIR-1's carry groups. Sums rows first, psum->sbuf
        # copies on the (now idle) ScalarE, per-half exp(-ln(s)) recips.
        pl = NPAIR - 1
        gA, gB = CARRY_G
        emit_mm2_chunk(pl, gA, KI_LO, combine=False)
        emit_mm2_chunk(pl, gA, KI_HI, combine=False)
        emit_mm2_srow(pl, gA)
        emit_mm2_chunk(pl, gB, KI_LO, combine=False)
        emit_mm2_chunk(pl, gB, KI_HI, combine=False)
        emit_mm2_srow(pl, gB)
        emit_mm2_ou(pl, gA, scalar_copy=True)
        emit_mm2_ou(pl, gB, scalar_copy=True)
        emit_recip_half(slice(0, 64))
        emit_recip_half(slice(64, 128))
        for g in (0, 1, 2, 3):
            emit_normalize(pl, g)

    return nc


def _shard_inputs(queries, keys, values):
    """Full [4,16,1024,64] fp32 -> per-core kq (bf16) / vext (bf16)."""
    import ml_dtypes

    q = np.ascontiguousarray(queries, dtype=np.float32).reshape(64, S, D)
    k = np.ascontiguousarray(keys, dtype=np.float32).reshape(64, S, D)
    v = np.ascontiguousarray(values, dtype=np.float32).reshape(64, S, D)

    qT = q.transpose(0, 2, 1)  # [64, D, S]
    kT = k.transpose(0, 2, 1)

    kq = np.empty((64 // 2, 128, 2 * S), ml_dtypes.bfloat16)
    kq[:, 0:64, 0:S] = qT[0::2]
    kq[:, 0:64, S:] = kT[0::2]
    kq[:, 64:128, 0:S] = qT[1::2]
    kq[:, 64:128, S:] = kT[1::2]

    vext = np.empty((64, 128, KT, 65), ml_dtypes.bfloat16)
    vext[..., 64] = 1.0
    vext[..., :64] = v.reshape(64, KT, 128, D).transpose(0, 2, 1, 3)

    sels = np.zeros((4, 128, D), np.float32)
    for r in range(4):
        sels[r, 32 * r, :] = 1.0

    in_maps = []
    for cc in range(N_CORES):
        in_maps.append(
            {
                "kq": np.ascontiguousarray(kq[cc * 4 : (cc + 1) * 4]),
                "vext": np.ascontiguousarray(vext[cc * 8 : (cc + 1) * 8]),
                "sels": sels,
            }
        )
    return in_maps


_CACHE = {}


def _get_nc(scale: float) -> bass.Bass:
    if scale not in _CACHE:
        _CACHE[scale] = build_nc(scale)
    return _CACHE[scale]


def run(queries, keys, values, d_k, trace=False, trace_kwargs=None):
    scale = float(1.0 / np.sqrt(np.float32(d_k)))
    nc = _get_nc(scale)
    in_maps = _shard_inputs(queries, keys, values)
    res = bass_utils.run_bass_kernel_spmd(
        nc,
        in_maps,
        core_ids=list(range(N_CORES)),
        trace=trace,
        **(trace_kwargs or {}),
    )
    outT = np.stack([r["outT"] for r in res.results])  # [8, 8, D, S]
    out = outT.reshape(64, D, S).transpose(0, 2, 1)  # [64, S, D]
    out = np.ascontiguousarray(out).reshape(4, 16, S, D).astype(np.float32)
    return out, res


def kernel(queries, keys, values, d_k):
    out, _ = run(queries, keys, values, d_k, trace=False)
    return out


# revision 20
# speedup vs baseline: 1.1625x; 1.1625x over previous
"""Dot-product attention on 8 Trainium2 NeuronCores.

Full inputs [B=4, H=16, S=1024, D=64] fp32. B*H = 64 heads are sharded
8-per-core (head parallel), processed in head PAIRS in a 3-deep software
pipeline: phase p runs MM1+exp of pair p, MM2 (+sums reciprocal) of pair
p-1, and normalize+store of pair p-2 concurrently.

Per head pair on-device:
  scores^T[k,q] = K d-major @ Q d-major  (bf16, K=64 contraction, the two
                                          heads row-packed into PE halves;
                                          2 ping-ponged [128,1024] PSUM
                                          stage tiles so the exp stream
                                          never stalls MM1)
  E = exp(scores^T / sqrt(d_k))          (ScalarE PSUM->SBUF, bf16 out,
                                          [128,1024] chunks — the ScalarE
                                          exp stream is the wall)
  outT+sums     = [V | 1]^T @ E          (bf16 SPLIT-K: even/odd k-halves
                                          as two concurrent K=64 row-group
                                          matmuls into separate PSUM banks,
                                          summed on DVE; ones column gives
                                          softmax sums in row 64)
  r = 1/sums                             (DVE reciprocal, scheduled a full
                                          phase ahead of its consumers; the
                                          tail pair uses ScalarE exp(-ln(s))
                                          since ScalarE is idle by then)
  out           = outT * (sel_r^T @ r)   (K=64 fp32r selector matmul
                                          broadcast + fused DVE multiply)
Host side transposes Q/K to d-major bf16 when sharding and un-transposes
the d-major output, both in numpy.

Toolchain notes for this container (walrus 2026-05-04 + bass_rust skew):
 - walrus accepts at most ONE sync-wait per instruction. A JSON pass over
   the BIR inserts NoOps carrying extra waits right before the owning
   instruction (same engine, in-order => semantics preserved). The
   TileContext tail drain is patched the same way.
"""

import json
from contextlib import ExitStack

import numpy as np

import concourse.bass as bass
import concourse.bass2jax as bass2jax
import concourse.mybir as mybir
import concourse.tile as tile
from concourse import bass_utils
from concourse.vector_clock import ScopedClock

F32 = mybir.dt.float32
F32R = mybir.dt.float32r
BF16 = mybir.dt.bfloat16

N_CORES = 8
HEADS_PER_CORE = 8
S = 1024
D = 64
KT = S // 128  # 8 k-tiles per head
NPAIR = HEADS_PER_CORE // 2

_DRAIN_MAX_WAITS = 1


def _split_drain_and_barrier(self, tick_clock, wait_clock):
    nc = self.nc
    drain_inst = nc.sync.drain()
    wait_clock.add_sem_waits(
        drain_inst.ins, ScopedClock({None: tick_clock.global_clock})
    )
    si = drain_inst.ins.sync_info
    if si is not None and si.on_wait and len(si.on_wait) > _DRAIN_MAX_WAITS:
        waits = list(si.on_wait)
        updates = list(si.on_update or [])
        drain_inst.ins.sync_info = mybir.SyncInfo(
            on_wait=waits[:_DRAIN_MAX_WAITS], on_update=[]
        )
        rest = waits[_DRAIN_MAX_WAITS:]
        for i in range(0, len(rest), _DRAIN_MAX_WAITS):
            extra = nc.sync.drain()
            extra.ins.sync_info = mybir.SyncInfo(
                on_wait=rest[i : i + _DRAIN_MAX_WAITS],
                on_update=updates if i + _DRAIN_MAX_WAITS >= len(rest) else [],
            )
    nc.all_engine_barrier()
    assert self.sems is not None
    popped = nc._tile_sem_poison_stack.pop()
    assert popped is self._sem_poison
    nc.clear_and_free_semaphores(list(self.sems.allocated().values()))
    nc.all_engine_barrier()


def _split_waits_in_bir(bir_json: bytes) -> bytes:
    """Hoist extra sync-waits onto NoOps inserted immediately before the
    owning instruction (same engine, in-order => semantics unchanged)."""
    j = json.loads(bir_json)
    n = 0
    for f in j["functions"]:
        for b in f["blocks"]:
            out = []
            for inst in b["instructions"]:
                si = inst.get("sync_info")
                waits = (si or {}).get("on_wait") or []
                if len(waits) > 1:
                    for w in waits[:-1]:
                        out.append(
                            {
                                "debug": inst.get("debug", 0),
                                "engine": inst["engine"],
                                "ins": [],
                                "outs": [],
                                "name": f"{inst['name']}-wsplit{n}",
                                "opcode": "NoOp",
                                "sync_info": {"on_update": [], "on_wait": [w]},
                            }
                        )
                        n += 1
                    si["on_wait"] = [waits[-1]]
                out.append(inst)
            b["instructions"] = out
    return json.dumps(j).encode()


_orig_compile_bir_kernel = bass_utils.compile_bir_kernel


def _compile_bir_kernel_splitting(bir_json, tmpdir, neff_name="file.neff"):
    return _orig_compile_bir_kernel(_split_waits_in_bir(bir_json), tmpdir, neff_name)


# walrus's lower_dve pass crashes on this kernel with ldw-opt enabled
ENABLE_LDW_OPT = False
_orig_run_command = bass_utils.run_command


def _run_command_ldw(argv, **kwargs):
    if ENABLE_LDW_OPT:
        argv = [
            a.replace("--enable-ldw-opt=false", "--enable-ldw-opt=true") for a in argv
        ]
    return _orig_run_command(argv, **kwargs)


def _install_patches():
    if not getattr(tile.TileContext, "_drain_split_installed", False):
        tile.TileContext._drain_and_barrier = _split_drain_and_barrier
        tile.TileContext._drain_split_installed = True
    if bass_utils.compile_bir_kernel is not _compile_bir_kernel_splitting:
        bass_utils.compile_bir_kernel = _compile_bir_kernel_splitting
        bass2jax.compile_bir_kernel = _compile_bir_kernel_splitting
        bass_utils.run_command = _run_command_ldw


# Tail reciprocal on ScalarE as exp(-ln(s)) (Ln and Exp share one act
# table set); mid-pipeline reciprocals run on DVE where they hide under
# the ScalarE exp stream.
TAIL_RECIP_SCALAR = True


def build_nc(scale: float) -> bass.Bass:
    _install_patches()
    nc = bass.Bass(
        trn_type="TRN2", target_bir_lowering=False, debug=False, num_devices=N_CORES
    )
    # kq[pair, 0:64, 0:1024] = Q^T head 2p ; [0:64, 1024:] = K^T head 2p
    # kq[pair, 64:128, ...]  = same for head 2p+1    (d-major, bf16)
    kq = nc.dram_tensor(
        "kq", [NPAIR, 128, 2 * S], BF16, kind="ExternalInput"
    ).ap()
    # vext[h, p, t, j]: V[h, 128*t + p, j] for j < 64, 1.0 at j == 64 (bf16)
    vext = nc.dram_tensor(
        "vext", [HEADS_PER_CORE, 128, KT, 65], BF16, kind="ExternalInput"
    ).ap()
    # sels[r, k, m] = 1.0 where k == 32*r: selector weights that extract and
    # broadcast row 32r of a [128, .] rhs across 64 output partitions.
    sels_d = nc.dram_tensor("sels", [4, 128, D], F32R, kind="ExternalInput").ap()
    outT = nc.dram_tensor(
        "outT", [HEADS_PER_CORE, D, S], F32, kind="ExternalOutput"
    ).ap()

    with tile.TileContext(nc) as tc, ExitStack() as ctx:
        sb = ctx.enter_context(tc.tile_pool(name="sb", bufs=2))
        singles = ctx.enter_context(tc.tile_pool(name="singles", bufs=1))
        ou_pool = ctx.enter_context(tc.tile_pool(name="ou", bufs=8))
        os_pool = ctx.enter_context(tc.tile_pool(name="os", bufs=2))
        # PSUM: 2 ping-pong stage tiles [128,1024] = 2 banks each (4 total);
        # "o" pool shares the other 4 banks between split-k MM2 A/B tiles
        # [65,512] and bcast tiles [64,512] (1 bank each).
        ps_stage = ctx.enter_context(
            tc.tile_pool(name="ps_stage", bufs=2, space="PSUM")
        )
        ps_o = ctx.enter_context(tc.tile_pool(name="ps_o", bufs=4, space="PSUM"))

        state = {}  # pair -> dict(kq, va, vb, e)
        nstate = {}  # pair -> dict(ou={g: tile}, os={half: tile})

        def prefetch(p, first=False):
            kq_s = sb.tile([128, 2 * S], BF16, tag="kq")
            if first:
                # Slots 0-7 run c=1, so Q[512:1024] + k-tile 0 -- one
                # CONTIGUOUS dma (wide lines = fewer descriptors) -- gates
                # the first matmul. K-tiles 1+ go on the scalar ring (k-tile
                # 1 alone first: needed ~1us after the first exp).
                nc.sync.dma_start(kq_s[:, 512 : S + 128], kq[p][:, 512 : S + 128])
                nc.sync.dma_start(
                    kq_s[:, S + 128 : S + 384], kq[p][:, S + 128 : S + 384]
                )
                nc.sync.dma_start(kq_s[:, S + 384 :], kq[p][:, S + 384 :])
                nc.sync.dma_start(kq_s[:, 0:512], kq[p][:, 0:512])
            else:
                nc.sync.dma_start(kq_s, kq[p])
            va = sb.tile([128, KT, 65], BF16, tag="va")
            nc.gpsimd.dma_start(va, vext[2 * p])
            vb = sb.tile([128, KT, 65], BF16, tag="vb")
            nc.gpsimd.dma_start(vb, vext[2 * p + 1])
            e_s = sb.tile([128, KT, 2, S], BF16, tag="e")
            state[p] = dict(kq=kq_s, va=va, vb=vb, e=e_s)

        def emit_mm1_exp(p, ki, c):
            st = state[p]
            kq_s = st["kq"]
            stage = ps_stage.tile([128, 1024], F32, tag="stage")
            kslice = slice(S + ki * 128, S + (ki + 1) * 128)
            qslice = slice(c * 512, (c + 1) * 512)
            for half in range(2):
                b = 64 * half
                nc.tensor.matmul(
                    stage[:, half * 512 : (half + 1) * 512],
                    kq_s[b : b + 64, kslice],
                    kq_s[b : b + 64, qslice],
                    start=True,
                    stop=True,
                )
            nc.scalar.activation(
                out=st["e"][:, ki, c, :],
                in_=stage,
                func=mybir.ActivationFunctionType.Exp,
                scale=scale,
            )

        mm2_ps = {}  # (p, g) -> (oa, ob)
        C_FIRST = 1  # c value run in slots 0-7 (chosen so the first-pair
        # critical dma [Q c=1 | k-tile 0] is one contiguous column range)

        def emit_mm2_chunk(p, g, kis, combine=True):
            """Part of a split-k MM2 group (kis = consecutive ki range).
            After the last chunk (if combine), emits the DVE combine. Carry
            groups do the sums rows FIRST so the reciprocal unblocks early."""
            half, c = divmod(g, 2)
            st = state[p]
            v_s = st["va"] if half == 0 else st["vb"]
            e_s = st["e"]
            if kis[0] == 0:
                oa = ps_o.tile([65, 512], F32, tag="o")
                ob = ps_o.tile([65, 512], F32, tag="o")
                mm2_ps[(p, g)] = (oa, ob)
            else:
                oa, ob = mm2_ps[(p, g)]
            qs = slice(half * 512, (half + 1) * 512)
            for ki in kis:
                nc.tensor.matmul(
                    oa, v_s[0:64, ki, :], e_s[0:64, ki, c, qs],
                    start=(ki == 0), stop=(ki == KT - 1),
                )
                nc.tensor.matmul(
                    ob, v_s[64:128, ki, :], e_s[64:128, ki, c, qs],
                    start=(ki == 0), stop=(ki == KT - 1),
                )
            if kis[-1] != KT - 1 or not combine:
                return
            emit_mm2_srow(p, g)
            emit_mm2_ou(p, g)

        def emit_mm2_srow(p, g):
            oa, ob = mm2_ps[(p, g)]
            srow = sums_sp[32 * g : 32 * g + 1, :]
            half, c = divmod(g, 2)
            if c != C_FIRST:
                nc.vector.tensor_copy(srow, oa[64:65, :])
                nc.vector.scalar_tensor_tensor(
                    out=srow, in0=ob[64:65, :], scalar=1.0,
                    op0=mybir.AluOpType.mult, in1=srow, op1=mybir.AluOpType.add,
                )

        def emit_mm2_ou(p, g, scalar_copy=False):
            oa, ob = mm2_ps[(p, g)]
            half, c = divmod(g, 2)
            ou = ou_pool.tile([65, 512], F32, tag="ou")
            if c != C_FIRST:
                # sums row already handled by emit_mm2_srow
                if scalar_copy:
                    nc.scalar.copy(ou[0:64, :], oa[0:64, :])
                else:
                    nc.vector.tensor_copy(ou[0:64, :], oa[0:64, :])
                nc.vector.scalar_tensor_tensor(
                    out=ou[0:64, :], in0=ob[0:64, :], scalar=1.0,
                    op0=mybir.AluOpType.mult, in1=ou[0:64, :],
                    op1=mybir.AluOpType.add,
                )
            else:
                srow = sums_sp[32 * g : 32 * g + 1, :]
                nc.vector.tensor_copy(ou, oa)
                nc.vector.scalar_tensor_tensor(
                    out=ou, in0=ob, scalar=1.0,
                    op0=mybir.AluOpType.mult, in1=ou, op1=mybir.AluOpType.add,
                )
                nc.vector.tensor_copy(srow, ou[64:65, :])
            nstate[p]["ou"][g] = ou

        def emit_recip(p):
            with nc.allow_low_precision(reason="fp32r recip for bcast matmul"):
                nc.vector.reciprocal(out=recip_sp, in_=sums_sp)

        def emit_recip_half(hr):
            # ScalarE is idle after the last exp: r = exp(-ln(s)) per half.
            with nc.allow_low_precision(reason="fp32r recip for bcast matmul"):
                nc.scalar.activation(
                    out=lntmp[hr, :], in_=sums_sp[hr, :],
                    func=mybir.ActivationFunctionType.Ln,
                )
                nc.scalar.activation(
                    out=recip_sp[hr, :], in_=lntmp[hr, :],
                    func=mybir.ActivationFunctionType.Exp, scale=-1.0,
                )

        def emit_normalize(p, g):
            half, c = divmod(g, 2)
            h = 2 * p + half
            hr = slice(0, 64) if g < 2 else slice(64, 128)
            bc = ps_o.tile([D, 512], F32, tag="o")
            nc.tensor.matmul(
                bc, sels_s[hr, g, :], recip_sp[hr, :], start=True, stop=True
            )
            if c == 0:
                o_s = os_pool.tile([D, S], F32, tag=f"os{half}")
                nstate[p]["os"][half] = o_s
            else:
                o_s = nstate[p]["os"][half]
            ou = nstate[p]["ou"][g]
            cs = slice(c * 512, (c + 1) * 512)
            nc.vector.scalar_tensor_tensor(
                out=o_s[:, cs],
                in0=bc,
                scalar=1.0,
                op0=mybir.AluOpType.mult,
                in1=ou[0:64, :],
                op1=mybir.AluOpType.mult,
            )
            nc.sync.dma_start(outT[h][:, cs], o_s[:, cs])

        prefetch(0, first=True)
        # constants are needed only from phase 1 on; issue them after kq(0)
        sels_s = singles.tile([128, 4, D], F32R, tag="sels")
        nc.gpsimd.dma_start(sels_s, sels_d.rearrange("r k m -> k r m"))
        # persistent sums/recip scratch; rows {0,32,64,96} hold live data,
        # the rest stay at 1.0 so the reciprocal never produces non-finites.
        sums_sp = singles.tile([128, 512], F32, tag="sums_sp")
        nc.vector.memset(sums_sp, 1.0)
        recip_sp = singles.tile([128, 512], F32R, tag="recip_sp")
        lntmp = singles.tile([128, 512], F32, tag="lntmp")

        # Phase p slot map (c-major: slots 0-7 = (ki, c=0), 8-15 = (ki, c=1)):
        #   slot 0-3:   MM2 of pair p-1's c=1 groups (g1, g3), half-group
        #               per slot to keep the PE queue smooth
        #   slot 4:     reciprocal of pair p-1's sums (8 slots of slack
        #               before its consumers at slots 12-15)
        #   slot 6:     prefetch pair p+1
        #   slot 8-11:  MM2 of pair p's own c=0 groups (g0, g2) -- their
        #               exps all landed in slots 0-7 of THIS phase
        #   slot 12-15: normalize pair p-1 (bcast+STT), out-DMA per half
        # groups g=(half,c): g0=(0,0) g1=(0,1) g2=(1,0) g3=(1,1)
        NORM_ORDER = (0, 2, 1, 3)
        KI_LO, KI_HI = range(0, 4), range(4, KT)
        # same-phase groups (c == C_FIRST) and carry groups (c != C_FIRST)
        SAME_G = (0 + C_FIRST, 2 + C_FIRST)
        CARRY_G = (1 - C_FIRST, 3 - C_FIRST)
        for p in range(NPAIR):
            nstate[p] = dict(ou={}, os={})
            for s in range(16):
                c = C_FIRST if s < 8 else 1 - C_FIRST
                ki = s % 8
                emit_mm1_exp(p, ki, c)
                if 1 <= p:
                    if s < 4:
                        g = CARRY_G[0] if s < 2 else CARRY_G[1]
                        emit_mm2_chunk(p - 1, g, KI_LO if s % 2 == 0 else KI_HI)
                    elif s == 4:
                        emit_recip(p - 1)
                    elif 12 <= s:
                        emit_normalize(p - 1, NORM_ORDER[s - 12])
                if 8 <= s < 12:
                    g = SAME_G[0] if s < 10 else SAME_G[1]
                    emit_mm2_chunk(p, g, KI_LO if s % 2 == 0 else KI_HI)
                if p + 1 < NPAIR and s == 6:
                    prefetch(p + 1)
        # tail: pair NPAIR-1's carry groups. Sums rows first, psum->sbuf
        # copies on the (now idle) ScalarE, per-half exp(-ln(s)) recips.
        pl = NPAIR - 1
        gA, gB = CARRY_G
        emit_mm2_chunk(pl, gA, KI_LO, combine=False)
        emit_mm2_chunk(pl, gA, KI_HI, combine=False)
        emit_mm2_srow(pl, gA)
        emit_mm2_chunk(pl, gB, KI_LO, combine=False)
        emit_mm2_chunk(pl, gB, KI_HI, combine=False)
        emit_mm2_srow(pl, gB)
        emit_mm2_ou(pl, gA, scalar_copy=True)
        emit_mm2_ou(pl, gB, scalar_copy=True)
        emit_recip_half(slice(0, 64))
        emit_recip_half(slice(64, 128))
        for g in (0, 1, 2, 3):
            emit_normalize(pl, g)

    return nc


def _shard_inputs(queries, keys, values):
    """Full [4,16,1024,64] fp32 -> per-core kq (bf16) / vext (bf16)."""
    import ml_dtypes

    q = np.ascontiguousarray(queries, dtype=np.float32).reshape(64, S, D)
    k = np.ascontiguousarray(keys, dtype=np.float32).reshape(64, S, D)
    v = np.ascontiguousarray(values, dtype=np.float32).reshape(64, S, D)

    qT = q.transpose(0, 2, 1)  # [64, D, S]
    kT = k.transpose(0, 2, 1)

    kq = np.empty((64 // 2, 128, 2 * S), ml_dtypes.bfloat16)
    kq[:, 0:64, 0:S] = qT[0::2]
    kq[:, 0:64, S:] = kT[0::2]
    kq[:, 64:128, 0:S] = qT[1::2]
    kq[:, 64:128, S:] = kT[1::2]

    vext = np.empty((64, 128, KT, 65), ml_dtypes.bfloat16)
    vext[..., 64] = 1.0
    vext[..., :64] = v.reshape(64, KT, 128, D).transpose(0, 2, 1, 3)

    sels = np.zeros((4, 128, D), np.float32)
    for r in range(4):
        sels[r, 32 * r, :] = 1.0

    in_maps = []
    for cc in range(N_CORES):
        in_maps.append(
            {
                "kq": np.ascontiguousarray(kq[cc * 4 : (cc + 1) * 4]),
                "vext": np.ascontiguousarray(vext[cc * 8 : (cc + 1) * 8]),
                "sels": sels,
            }
        )
    return in_maps


_CACHE = {}


def _get_nc(scale: float) -> bass.Bass:
    if scale not in _CACHE:
        _CACHE[scale] = build_nc(scale)
    return _CACHE[scale]


def run(queries, keys, values, d_k, trace=False, trace_kwargs=None):
    scale = float(1.0 / np.sqrt(np.float32(d_k)))
    nc = _get_nc(scale)
    in_maps = _shard_inputs(queries, keys, values)
    res = bass_utils.run_bass_kernel_spmd(
        nc,
        in_maps,
        core_ids=list(range(N_CORES)),
        trace=trace,
        **(trace_kwargs or {}),
    )
    outT = np.stack([r["outT"] for r in res.results])  # [8, 8, D, S]
    out = outT.reshape(64, D, S).transpose(0, 2, 1)  # [64, S, D]
    out = np.ascontiguousarray(out).reshape(4, 16, S, D).astype(np.float32)
    return out, res


def kernel(queries, keys, values, d_k):
    out, _ = run(queries, keys, values, d_k, trace=False)
    return out
